# revision 1
# baseline (speedup 1.0000x reference)
"""GQA attention kernel for Trainium2 (8 NeuronCores).

Problem: B=2, S=2048, D=2048, H=16 heads of DH=128, KV=4 kv heads, G=4
query heads per kv head.  Full (dense) attention, fp32 I/O.

Sharding: batch (2) x kv-head (4) = 8 cores, zero redundant FLOPs.
Each core computes, for its (batch b, kv head h):
    Q_g = x_b @ Wq[:, h,g]  (4 query heads), K = x_b @ Wk[:, h],
    V = x_b @ Wv[:, h], O_g = softmax(Q_g K^T / sqrt(DH)) V,
    y_partial = concat_g(O_g) @ Wo[h-rows, :]
Host sums the 4 kv-head partials per batch and adds bo.

On-chip strategy:
 - Projections and out-proj run as residual-fp8 DoubleRow matmuls:
   each operand is split (on host for x/W, on chip for O) into
   e4m3 hi + e4m3 lo residual; products hi.hi + hi.lo + lo.hi are kept
   (lo.lo dropped).  3 DoubleRow matmuls per 256-deep contraction pair
   = 1.5 PE cycles/row vs bf16's 2.0, at better-than-bf16 accuracy.
 - Scores S^T tiles ([k, q], lhsT=KT slice, rhs=QT block) and AV
   (lhsT=V tile, rhs=exp tile) in bf16.  1/sqrt(DH) is applied inside
   the exp activation (scale operand), keeping qt/kt at unit scale.
 - exp of the 16 score k-tiles per (g, q-block): 12 tiles on the Act
   engine (native Exp, PSUM pair reads [128,1024]), 4 tiles as
   Schraudolph bit-trick exponentials on DVE/Pool (tensor_scalar
   fp32->int16 of s*A+B, bitcast to bf16), spreading exp across three
   engines so the PE stays the bottleneck.
 - rowsum via DVE pairwise tree-add of exp tiles + one [128,1]-ones
   matmul; reciprocal on DVE; 1/r broadcast on Pool; normalization
   multiply on DVE produces O*16/r fp32, split into e4m3 hi/lo for the
   residual out-proj (Act copy + Pool subtract).
 - y written bf16 (PSUM * 1/1024 scale), host sums partials in fp32.
"""

import sys

if "/opt/trn_rl_repo" not in sys.path:
    sys.path.insert(0, "/opt/trn_rl_repo")

import numpy as np
import ml_dtypes
from contextlib import ExitStack

B, S, D = 2, 2048, 2048
H, DH, GRP = 16, 128, 4
KV = H // GRP            # 4 kv heads
EH = GRP * DH            # 512 = query-head columns per kv head
SCALE = float(1.0 / np.sqrt(np.float32(DH)))
P = 128                  # partitions
NB = 512                 # matmul moving-dim block (one PSUM bank fp32)
WSC = 64.0               # weight fp8 pre-scale
OSC = 16.0               # ot fp8 pre-scale

# Schraudolph exp-approx constants (bf16 bit domain), folding in SCALE
SCH_A = float(128.0 * SCALE / np.log(2.0))
SCH_B = float((127.0 - 0.0579) * 128.0)


def _emit(ctx, tc, aps, s=S, d=D):
    import concourse.bass as bass
    from concourse import mybir

    nc = tc.nc
    bf16 = mybir.dt.bfloat16
    f32 = mybir.dt.float32
    e4 = mybir.dt.float8e4
    i16 = mybir.dt.int16
    DR = mybir.MatmulPerfMode.DoubleRow
    Exp = mybir.ActivationFunctionType.Exp
    Identity = mybir.ActivationFunctionType.Identity

    nt = s // P           # 128-tiles along s
    nd = d // P           # 128-tiles along d (contraction)
    npr = nd // 2         # 256-pairs along d
    nsb = s // NB         # 512-blocks along s
    ndb = d // NB         # 512-blocks along d (out columns)

    persist = ctx.enter_context(tc.tile_pool(name="persist", bufs=1))

    # ---- persistent tiles ----
    wohi_sb = persist.tile([P, GRP, d], e4)
    wolo_sb = persist.tile([P, GRP, d], e4)
    qt_sb = persist.tile([P, GRP, s], bf16)
    kt_sb = persist.tile([P, s], bf16)
    v_sb = persist.tile([P, nt, DH], bf16)
    ot8hi = persist.tile([P, GRP, s], e4)
    ot8lo = persist.tile([P, GRP, s], e4)
    bq_sb = persist.tile([P, GRP], f32)
    bk_sb = persist.tile([P, 1], f32)
    bvb_sb = persist.tile([P, DH], f32)
    zbias = persist.tile([P, 1], f32)
    ones16 = persist.tile([P, 1], bf16)

    nc.vector.memset(ones16, 1.0 / OSC)
    nc.vector.memset(zbias, 0.0)

    # ================= phase P: projections =================
    projpool = tc.tile_pool(name="projp", bufs=1)
    projp = projpool.__enter__()
    psP = tc.tile_pool(name="psP", bufs=2, space="PSUM")
    psPp = psP.__enter__()

    xhi_sb = projp.tile([P, nd, s], e4)
    xlo_sb = projp.tile([P, nd, s], e4)
    wqhi_sb = projp.tile([P, nd, EH], e4)
    wqlo_sb = projp.tile([P, nd, EH], e4)
    wkhi_sb = projp.tile([P, nd, DH], e4)
    wklo_sb = projp.tile([P, nd, DH], e4)
    wvhi_sb = projp.tile([P, nd, DH], e4)
    wvlo_sb = projp.tile([P, nd, DH], e4)

    # loads: weights + biases first (gate the first Q/K/V blocks), then x
    # hi/lo in s-column chunks (sb-major) so proj blocks unblock per sb,
    # wo last (only needed in phase O).
    nc.sync.dma_start(out=wqhi_sb, in_=aps["wqhi"].rearrange("(t p) e -> p t e", p=P))
    nc.sync.dma_start(out=wqlo_sb, in_=aps["wqlo"].rearrange("(t p) e -> p t e", p=P))
    nc.sync.dma_start(out=bq_sb, in_=aps["bq"].rearrange("(g p) -> p g", p=P))
    nc.sync.dma_start(out=bk_sb, in_=aps["bk"].rearrange("(p o) -> p o", o=1))
    bv = aps["bv"]
    bv_bcast = bass.AP(tensor=bv.tensor, offset=bv.offset,
                       ap=[[0, P]] + list(bv.ap))
    nc.sync.dma_start(out=bvb_sb, in_=bv_bcast)
    nc.sync.dma_start(out=wkhi_sb, in_=aps["wkhi"].rearrange("(t p) e -> p t e", p=P))
    nc.sync.dma_start(out=wklo_sb, in_=aps["wklo"].rearrange("(t p) e -> p t e", p=P))
    nc.sync.dma_start(out=wvhi_sb, in_=aps["wvhi"].rearrange("(t p) e -> p t e", p=P))
    nc.sync.dma_start(out=wvlo_sb, in_=aps["wvlo"].rearrange("(t p) e -> p t e", p=P))
    xhi_r = aps["xhi"].rearrange("(t p) s -> p t s", p=P)
    xlo_r = aps["xlo"].rearrange("(t p) s -> p t s", p=P)
    for sb in range(nsb):
        ssl = slice(sb * NB, (sb + 1) * NB)
        for t in range(nd):
            nc.sync.dma_start(out=xhi_sb[:, t, ssl], in_=xhi_r[:, t, ssl])
            nc.sync.dma_start(out=xlo_sb[:, t, ssl], in_=xlo_r[:, t, ssl])
    nc.sync.dma_start(out=wohi_sb, in_=aps["wohi"].rearrange("(g p) d -> p g d", p=P))
    nc.sync.dma_start(out=wolo_sb, in_=aps["wolo"].rearrange("(g p) d -> p g d", p=P))

    def res_mm(ps, w_hi, w_lo, x_hi, x_lo, pr, first, last):
        """3 DoubleRow matmuls for one 256-deep pair: hi.hi + hi.lo + lo.hi.
        w_* are lhsT [P, 2, m] APs, x_* are rhs [P, 2, n] APs."""
        nc.tensor.matmul(ps, lhsT=w_hi, rhs=x_hi, start=first, stop=False,
                         perf_mode=DR)
        nc.tensor.matmul(ps, lhsT=w_lo, rhs=x_hi, start=False, stop=False,
                         perf_mode=DR)
        nc.tensor.matmul(ps, lhsT=w_hi, rhs=x_lo, start=False, stop=last,
                         perf_mode=DR)

    # sb-major so each s-block's Q/K/V runs as soon as its x columns land
    for sb in range(nsb):
        ssl = slice(sb * NB, (sb + 1) * NB)
        # Q blocks: QT[dh, s] per g
        for g in range(GRP):
            gsl = slice(g * DH, (g + 1) * DH)
            ps = psPp.tile([P, NB], f32, tag="mm")
            for pr in range(npr):
                jsl = slice(2 * pr, 2 * pr + 2)
                res_mm(ps, wqhi_sb[:, jsl, gsl], wqlo_sb[:, jsl, gsl],
                       xhi_sb[:, jsl, ssl], xlo_sb[:, jsl, ssl],
                       pr, pr == 0, pr == npr - 1)
            nc.scalar.activation(out=qt_sb[:, g, ssl], in_=ps, func=Identity,
                                 bias=bq_sb[:, g:g + 1], scale=1.0 / WSC)
        # K block: KT[dh, s]
        ps = psPp.tile([P, NB], f32, tag="mm")
        for pr in range(npr):
            jsl = slice(2 * pr, 2 * pr + 2)
            res_mm(ps, wkhi_sb[:, jsl, :], wklo_sb[:, jsl, :],
                   xhi_sb[:, jsl, ssl], xlo_sb[:, jsl, ssl],
                   pr, pr == 0, pr == npr - 1)
        nc.scalar.activation(out=kt_sb[:, ssl], in_=ps, func=Identity,
                             bias=bk_sb[:, 0:1], scale=1.0 / WSC)
        # V tiles: V[k, dh] natural (x stationary, wv moving)
        for st in range(4 * sb, 4 * sb + 4):
            tsl = slice(st * P, (st + 1) * P)
            ps = psPp.tile([P, NB], f32, tag="mm")
            for pr in range(npr):
                jsl = slice(2 * pr, 2 * pr + 2)
                res_mm(ps[:, 0:DH], xhi_sb[:, jsl, tsl], xlo_sb[:, jsl, tsl],
                       wvhi_sb[:, jsl, :], wvlo_sb[:, jsl, :],
                       pr, pr == 0, pr == npr - 1)
            nc.vector.scalar_tensor_tensor(
                out=v_sb[:, st, :], in0=ps[:, 0:DH], scalar=1.0 / WSC,
                in1=bvb_sb, op0=mybir.AluOpType.mult, op1=mybir.AluOpType.add)

    projpool.__exit__(None, None, None)
    psP.__exit__(None, None, None)

    # ================= phase A: attention =================
    psSpool = tc.tile_pool(name="psS", bufs=2, space="PSUM")
    psS = psSpool.__enter__()
    psOApool = tc.tile_pool(name="psOA", bufs=3, space="PSUM")
    psOA = psOApool.__enter__()
    psRpool = tc.tile_pool(name="psR", bufs=1, space="PSUM")
    psR = psRpool.__enter__()
    ptpool = ctx.enter_context(tc.tile_pool(name="ptp", bufs=2))
    trpool = ctx.enter_context(tc.tile_pool(name="trp", bufs=2))
    scpool = ctx.enter_context(tc.tile_pool(name="scp", bufs=2))

    npair = nt // 2
    blocks = [(g, qb) for g in range(GRP) for qb in range(nsb)]

    def emit_tree(tree, tall, acc512):
        """Finish a block's rowsum tree: combine the 4 level-1 pair-sums and
        fold [P,2,NB] -> [P,NB]."""
        nc.vector.tensor_add(tall, tree[0], tree[1])
        nc.vector.tensor_add(tall, tall, tree[2])
        nc.vector.tensor_add(tall, tall, tree[3])
        nc.vector.tensor_add(acc512, tall[:, 0, :], tall[:, 1, :])

    def finish_norm(pg, pqb, pacc, pps_o):
        """Rowsum matmul + normalize + fp8 hi/lo split for a finished block.
        PE: 1 ones-matmul; DVE: recip + mul; Pool: bcast + hi copy + lo sub."""
        pqsl = slice(pqb * NB, (pqb + 1) * NB)
        ps_r = psR.tile([1, NB], f32, tag="r")
        nc.tensor.matmul(ps_r, lhsT=ones16[:, 0:1], rhs=pacc,
                         start=True, stop=True)
        rrow = scpool.tile([1, NB], f32, tag="rrow")
        nc.vector.reciprocal(rrow, ps_r)
        rb = scpool.tile([P, NB], f32, tag="rb")
        nc.gpsimd.partition_broadcast(rb, rrow[0:1, :])
        otf = scpool.tile([P, NB], f32, tag="otf")
        nc.vector.tensor_mul(otf, pps_o, rb)
        nc.gpsimd.tensor_copy(ot8hi[:, pg, pqsl], otf)
        nc.gpsimd.tensor_sub(ot8lo[:, pg, pqsl], otf, ot8hi[:, pg, pqsl])

    def emit_block(g, qb, prev):
        """Emit one (g, q-block): scores+exp for this block interleaved on PE
        with AV of `prev`; rowsum/normalize of `prev` rides along at the end.
        Returns this block's state tuple."""
        qsl = slice(qb * NB, (qb + 1) * NB)
        pt = ptpool.tile([P, nt, NB], bf16, tag="pt")
        ps_o = psOA.tile([P, NB], f32, tag="o")
        tree = [trpool.tile([P, 2, NB], bf16, tag=f"t{i}", name=f"tree{i}")
                for i in range(4)]
        tall = trpool.tile([P, 2, NB], bf16, tag="tall")
        acc512 = trpool.tile([P, NB], bf16, tag="acc")
        if prev is not None:
            (pg, pqb, ppt, pps_o, ptree, ptall, pacc) = prev

        for p in range(npair):
            ps_s = psS.tile([P, 2, NB], f32, tag="s")
            for j in (0, 1):
                ki = 2 * p + j
                nc.tensor.matmul(
                    ps_s[:, j, :], lhsT=kt_sb[:, ki * P:(ki + 1) * P],
                    rhs=qt_sb[:, g, qsl], start=True, stop=True)
            # exp of the pair: Act p0-5 native, DVE p6 Schraudolph, p7 split
            # Act/DVE (GPSIMD cannot read PSUM, so Pool gets no exp work)
            if p < 6:
                nc.scalar.activation(
                    out=pt[:, 2 * p:2 * p + 2, :], in_=ps_s, func=Exp,
                    bias=zbias[:, 0:1], scale=SCALE)
            elif p == 6:
                nc.vector.tensor_scalar(
                    out=pt[:, 12:14, :].bitcast(i16), in0=ps_s,
                    scalar1=SCH_A, scalar2=SCH_B,
                    op0=mybir.AluOpType.mult, op1=mybir.AluOpType.add)
            else:
                nc.scalar.activation(
                    out=pt[:, 14:15, :], in_=ps_s[:, 0, :], func=Exp,
                    bias=zbias[:, 0:1], scale=SCALE)
                nc.vector.tensor_scalar(
                    out=pt[:, 15:16, :].bitcast(i16), in0=ps_s[:, 1, :],
                    scalar1=SCH_A, scalar2=SCH_B,
                    op0=mybir.AluOpType.mult, op1=mybir.AluOpType.add)
            # pairwise rowsum tree level 1 as tiles become ready
            if p % 2 == 1:
                nc.vector.tensor_add(tree[p // 2], pt[:, 2 * p - 2:2 * p, :],
                                     pt[:, 2 * p:2 * p + 2, :])
            # AV of prev interleaves with this block's scores on PE
            if prev is not None:
                nc.tensor.matmul(
                    pps_o, lhsT=v_sb[:, 2 * p, :], rhs=ppt[:, 2 * p, :],
                    start=(p == 0), stop=False)
                nc.tensor.matmul(
                    pps_o, lhsT=v_sb[:, 2 * p + 1, :], rhs=ppt[:, 2 * p + 1, :],
                    start=False, stop=(p == npair - 1))
        # fold THIS block's tree right after its last level-1 add so the DVE
        # has pacc ready well before next block's end-of-stream ones-matmul
        emit_tree(tree, tall, acc512)
        if prev is not None:
            finish_norm(pg, pqb, pacc, pps_o)
        return (g, qb, pt, ps_o, tree, tall, acc512)

    prev = None
    for g, qb in blocks:
        prev = emit_block(g, qb, prev)

    # epilogue: drain last block (AV + rowsum + normalize); its tree was
    # already folded inside emit_block
    (pg, pqb, ppt, pps_o, ptree, ptall, pacc) = prev
    for ki in range(nt):
        nc.tensor.matmul(pps_o, lhsT=v_sb[:, ki, :], rhs=ppt[:, ki, :],
                         start=(ki == 0), stop=(ki == nt - 1))
    finish_norm(pg, pqb, pacc, pps_o)
    psRpool.__exit__(None, None, None)
    psOApool.__exit__(None, None, None)
    psSpool.__exit__(None, None, None)

    # ================= phase O: out-projection =================
    psO = ctx.enter_context(tc.tile_pool(name="psO", bufs=4, space="PSUM"))
    ypool = ctx.enter_context(tc.tile_pool(name="yp", bufs=3))
    y = aps["y"]
    ngp = GRP // 2
    for st in range(nt):
        tsl = slice(st * P, (st + 1) * P)
        y_sb = ypool.tile([P, ndb, NB], bf16, tag="y")
        for db in range(ndb):
            dsl = slice(db * NB, (db + 1) * NB)
            ps_y = psO.tile([P, NB], f32, tag="y")
            for gp in range(ngp):
                jsl = slice(2 * gp, 2 * gp + 2)
                res_mm(ps_y, ot8hi[:, jsl, tsl], ot8lo[:, jsl, tsl],
                       wohi_sb[:, jsl, dsl], wolo_sb[:, jsl, dsl],
                       gp, gp == 0, gp == ngp - 1)
            if db % 2 == 0:
                nc.scalar.mul(y_sb[:, db, :], ps_y, 1.0 / (OSC * WSC))
            else:
                nc.vector.tensor_scalar_mul(y_sb[:, db, :], ps_y, 1.0 / (OSC * WSC))
        nc.sync.dma_start(out=y[tsl, :], in_=y_sb)


def build_program(s=S, d=D):
    import concourse.tile as tile
    from concourse import bacc, mybir

    nc = bacc.Bacc("TRN2", target_bir_lowering=False, debug=False)
    bf16 = mybir.dt.bfloat16
    f32 = mybir.dt.float32
    e4 = mybir.dt.float8e4
    names = {
        "xhi": ([d, s], e4), "xlo": ([d, s], e4),
        "wqhi": ([d, EH], e4), "wqlo": ([d, EH], e4),
        "wkhi": ([d, DH], e4), "wklo": ([d, DH], e4),
        "wvhi": ([d, DH], e4), "wvlo": ([d, DH], e4),
        "wohi": ([EH, d], e4), "wolo": ([EH, d], e4),
        "bq": ([EH], f32), "bk": ([DH], f32), "bv": ([DH], f32),
    }
    aps = {k: nc.dram_tensor(k, sh, dt, kind="ExternalInput").ap()
           for k, (sh, dt) in names.items()}
    aps["y"] = nc.dram_tensor("y", [s, d], bf16, kind="ExternalOutput").ap()
    with tile.TileContext(nc) as tc:
        with ExitStack() as ctx:
            _emit(ctx, tc, aps, s=s, d=d)
    nc.compile()
    return nc


def _res_split(v32, scale):
    """fp8 residual split of v32*scale: hi = e4(v*scale), lo = e4(v*scale-hi)."""
    e4 = ml_dtypes.float8_e4m3
    vs = v32 * np.float32(scale)
    hi = vs.astype(e4)
    lo = (vs - hi.astype(np.float32)).astype(e4)
    return hi, lo


def make_in_maps(x, Wq, bq, Wk, bk, Wv, bv, Wo, bo):
    in_maps = []
    xsplit = []
    for b in range(B):
        xT = np.ascontiguousarray(x[b].T)  # [D, S]
        xsplit.append(_res_split(xT, 1.0))
    for b in range(B):
        xhi, xlo = xsplit[b]
        for h in range(KV):
            wqh, wql = _res_split(np.ascontiguousarray(
                Wq[:, h * EH:(h + 1) * EH]), WSC)
            wkh, wkl = _res_split(np.ascontiguousarray(
                Wk[:, h * DH:(h + 1) * DH]), WSC)
            wvh, wvl = _res_split(np.ascontiguousarray(
                Wv[:, h * DH:(h + 1) * DH]), WSC)
            woh, wol = _res_split(np.ascontiguousarray(
                Wo[h * EH:(h + 1) * EH, :]), WSC)
            in_maps.append({
                "xhi": xhi, "xlo": xlo,
                "wqhi": wqh, "wqlo": wql,
                "wkhi": wkh, "wklo": wkl,
                "wvhi": wvh, "wvlo": wvl,
                "wohi": woh, "wolo": wol,
                "bq": np.ascontiguousarray(bq[h * EH:(h + 1) * EH]).astype(np.float32),
                "bk": np.ascontiguousarray(bk[h * DH:(h + 1) * DH]).astype(np.float32),
                "bv": np.ascontiguousarray(bv[h * DH:(h + 1) * DH]).astype(np.float32),
            })
    return in_maps


_PROG = None


def _get_program():
    global _PROG
    if _PROG is None:
        _PROG = build_program()
    return _PROG


def run_cores(in_maps, trace=False, **kw):
    from concourse.bass_utils import run_bass_kernel_spmd
    nc = _get_program()
    return run_bass_kernel_spmd(nc, in_maps, list(range(8)), trace=trace, **kw)


def kernel(**inputs):
    x = np.asarray(inputs["x"], dtype=np.float32)
    Wq = np.asarray(inputs["Wq"], dtype=np.float32)
    bq = np.asarray(inputs["bq"], dtype=np.float32)
    Wk = np.asarray(inputs["Wk"], dtype=np.float32)
    bk = np.asarray(inputs["bk"], dtype=np.float32)
    Wv = np.asarray(inputs["Wv"], dtype=np.float32)
    bv = np.asarray(inputs["bv"], dtype=np.float32)
    Wo = np.asarray(inputs["Wo"], dtype=np.float32)
    bo = np.asarray(inputs["bo"], dtype=np.float32)

    in_maps = make_in_maps(x, Wq, bq, Wk, bk, Wv, bv, Wo, bo)
    res = run_cores(in_maps)
    out = np.empty((B, S, D), dtype=np.float32)
    for b in range(B):
        acc = res.results[b * KV]["y"].astype(np.float32)
        for h in range(1, KV):
            acc = acc + res.results[b * KV + h]["y"].astype(np.float32)
        out[b] = acc + bo[None, :]
    return out



# revision 6
# speedup vs baseline: 1.1263x; 1.1263x over previous
"""GQA attention kernel for Trainium2 (8 NeuronCores).

Problem: B=2, S=2048, D=2048, H=16 heads of DH=128, KV=4 kv heads, G=4
query heads per kv head.  Full (dense) attention, fp32 I/O.

Sharding: batch (2) x kv-head (4) = 8 cores, zero redundant FLOPs.
Each core computes, for its (batch b, kv head h):
    Q_g = x_b @ Wq[:, h,g]  (4 query heads), K = x_b @ Wk[:, h],
    V = x_b @ Wv[:, h], O_g = softmax(Q_g K^T / sqrt(DH)) V,
    y_partial = concat_g(O_g) @ Wo[h-rows, :]
Host sums the 4 kv-head partials per batch and adds bo.

On-chip strategy:
 - Projections and out-proj run as residual-fp8 DoubleRow matmuls:
   each operand is split (on host for x/W, on chip for O) into
   e4m3 hi + e4m3 lo residual; products hi.hi + hi.lo + lo.hi are kept
   (lo.lo dropped).  3 DoubleRow matmuls per 256-deep contraction pair
   = 1.5 PE cycles/row vs bf16's 2.0, at better-than-bf16 accuracy.
 - hi/lo pairs are PACKED into single DRAM tensors host-side so each
   logical load is ONE dma_start (HWDGE dispatch is ~630ns serialized;
   the old per-(sb,t) x chunking burned ~80us of dispatch).
 - Scores S^T tiles ([k, q], lhsT=KT slice, rhs=QT block) and AV
   (lhsT=V tile, rhs=exp tile) in bf16.  1/sqrt(DH) is applied inside
   the exp activation (scale operand), keeping qt/kt at unit scale.
 - exp of the 16 score k-tiles per (g, q-block): 12 tiles on the Act
   engine (native Exp, PSUM pair reads [128,1024]), 4 tiles as
   Schraudolph bit-trick exponentials on DVE (tensor_scalar
   fp32->int16 of s*A+B, bitcast to bf16), spreading exp across
   engines so the PE stays the bottleneck.
 - rowsum via DVE pairwise tree-add of exp tiles; the last tree level
   emits e4m3 partial sums so the final partition-reduce is a single
   fp8 DoubleRow ones-matmul (256 PE cycles vs 512); reciprocal on
   DVE; 1/r broadcast on Pool; normalization multiply on DVE produces
   O*16/r fp32, split into e4m3 hi/lo for the residual out-proj
   (Pool copy + Pool subtract).
 - y written bf16 (PSUM * 1/1024 scale), host sums partials in fp32.
"""

import sys

if "/opt/trn_rl_repo" not in sys.path:
    sys.path.insert(0, "/opt/trn_rl_repo")

import numpy as np
import ml_dtypes
from contextlib import ExitStack

B, S, D = 2, 2048, 2048
H, DH, GRP = 16, 128, 4
KV = H // GRP            # 4 kv heads
EH = GRP * DH            # 512 = query-head columns per kv head
SCALE = float(1.0 / np.sqrt(np.float32(DH)))
P = 128                  # partitions
NB = 512                 # matmul moving-dim block (one PSUM bank fp32)
WSC = 64.0               # weight fp8 pre-scale
OSC = 16.0               # ot fp8 pre-scale

# Schraudolph exp-approx constants (bf16 bit domain), folding in SCALE
SCH_A = float(128.0 * SCALE / np.log(2.0))
SCH_B = float((127.0 - 0.0579) * 128.0)


def _emit(ctx, tc, aps, s=S, d=D):
    import concourse.bass as bass
    from concourse import mybir

    nc = tc.nc
    bf16 = mybir.dt.bfloat16
    f32 = mybir.dt.float32
    e4 = mybir.dt.float8e4
    i16 = mybir.dt.int16
    DR = mybir.MatmulPerfMode.DoubleRow
    Exp = mybir.ActivationFunctionType.Exp
    Identity = mybir.ActivationFunctionType.Identity

    nt = s // P           # 128-tiles along s
    nd = d // P           # 128-tiles along d (contraction)
    npr = nd // 2         # 256-pairs along d
    nsb = s // NB         # 512-blocks along s
    ndb = d // NB         # 512-blocks along d (out columns)

    persist = ctx.enter_context(tc.tile_pool(name="persist", bufs=1))

    # ---- persistent tiles ----
    wo_sb = persist.tile([P, GRP, 2, d], e4)      # [.,g,hi/lo,d]
    qt_sb = persist.tile([P, GRP, s], bf16)
    kt_sb = persist.tile([P, s], bf16)
    v_sb = persist.tile([P, nt, DH], bf16)
    ot8hi = persist.tile([P, GRP, s], e4)
    ot8lo = persist.tile([P, GRP, s], e4)
    bq_sb = persist.tile([P, GRP], f32)
    bk_sb = persist.tile([P, 1], f32)
    bvb_sb = persist.tile([P, DH], f32)
    zbias = persist.tile([P, 1], f32)
    ones16 = persist.tile([P, 1], bf16)

    nc.vector.memset(ones16, 1.0 / OSC)
    nc.vector.memset(zbias, 0.0)

    # ================= phase P: projections =================
    projpool = tc.tile_pool(name="projp", bufs=1)
    projp = projpool.__enter__()
    psP = tc.tile_pool(name="psP", bufs=2, space="PSUM")
    psPp = psP.__enter__()

    xp_sb = projp.tile([P, nd, nsb, 2, NB], e4)   # [.,t,sb,hi/lo,n]
    wq_sb = projp.tile([P, nd, 2, EH], e4)        # [.,t,hi/lo,e]
    wkv_sb = projp.tile([P, nd, 4, DH], e4)       # [.,t,khi/klo/vhi/vlo,dh]

    # loads, ordered for earliest PE start (HWDGE dispatch is serial):
    # wkv first (gates K/V of sb0), then x sb0 split in two t-halves
    # (prs start as soon as the first half lands), biases, wq (only
    # needed ~10us in, when Q blocks start), x sb1-3, wo last.
    xp_r = aps["xp"].rearrange("(t p) b h n -> p t b h n", p=P)
    wkv_r = aps["wkv"].rearrange("(t p) c e -> p t c e", p=P)
    nc.sync.dma_start(out=wkv_sb, in_=wkv_r)
    nc.sync.dma_start(out=xp_sb[:, 0:nd // 2, 0, :, :],
                      in_=xp_r[:, 0:nd // 2, 0, :, :])
    nc.sync.dma_start(out=bq_sb, in_=aps["bq"].rearrange("(g p) -> p g", p=P))
    nc.sync.dma_start(out=bk_sb, in_=aps["bk"].rearrange("(p o) -> p o", o=1))
    bv = aps["bv"]
    bv_bcast = bass.AP(tensor=bv.tensor, offset=bv.offset,
                       ap=[[0, P]] + list(bv.ap))
    nc.sync.dma_start(out=bvb_sb, in_=bv_bcast)
    nc.sync.dma_start(out=xp_sb[:, nd // 2:, 0, :, :],
                      in_=xp_r[:, nd // 2:, 0, :, :])
    nc.sync.dma_start(out=wq_sb, in_=aps["wq"].rearrange("(t p) h e -> p t h e", p=P))
    for sb in range(1, nsb):
        nc.sync.dma_start(out=xp_sb[:, :, sb, :, :], in_=xp_r[:, :, sb, :, :])
    nc.sync.dma_start(out=wo_sb, in_=aps["wo"].rearrange("(g p) h d -> p g h d", p=P))

    def res_mm(ps, w_hi, w_lo, x_hi, x_lo, pr, first, last):
        """3 DoubleRow matmuls for one 256-deep pair: hi.hi + hi.lo + lo.hi.
        w_* are lhsT [P, 2, m] APs, x_* are rhs [P, 2, n] APs."""
        nc.tensor.matmul(ps, lhsT=w_hi, rhs=x_hi, start=first, stop=False,
                         perf_mode=DR)
        nc.tensor.matmul(ps, lhsT=w_lo, rhs=x_hi, start=False, stop=False,
                         perf_mode=DR)
        nc.tensor.matmul(ps, lhsT=w_hi, rhs=x_lo, start=False, stop=last,
                         perf_mode=DR)

    # sb-major so each s-block's K/V/Q runs as soon as its x columns land;
    # K first within the sb (kt completeness gates the attention phase).
    for sb in range(nsb):
        ssl = slice(sb * NB, (sb + 1) * NB)
        # K block: KT[dh, s]
        ps = psPp.tile([P, NB], f32, tag="mm")
        for pr in range(npr):
            jsl = slice(2 * pr, 2 * pr + 2)
            res_mm(ps, wkv_sb[:, jsl, 0, :], wkv_sb[:, jsl, 1, :],
                   xp_sb[:, jsl, sb, 0, :], xp_sb[:, jsl, sb, 1, :],
                   pr, pr == 0, pr == npr - 1)
        nc.scalar.activation(out=kt_sb[:, ssl], in_=ps, func=Identity,
                             bias=bk_sb[:, 0:1], scale=1.0 / WSC)
        # V tiles: V[k, dh] natural (x stationary, wv moving)
        for st in range(4 * sb, 4 * sb + 4):
            csl = slice((st % 4) * P, (st % 4 + 1) * P)
            ps = psPp.tile([P, NB], f32, tag="mm")
            for pr in range(npr):
                jsl = slice(2 * pr, 2 * pr + 2)
                res_mm(ps[:, 0:DH],
                       xp_sb[:, jsl, sb, 0, csl], xp_sb[:, jsl, sb, 1, csl],
                       wkv_sb[:, jsl, 2, :], wkv_sb[:, jsl, 3, :],
                       pr, pr == 0, pr == npr - 1)
            nc.vector.scalar_tensor_tensor(
                out=v_sb[:, st, :], in0=ps[:, 0:DH], scalar=1.0 / WSC,
                in1=bvb_sb, op0=mybir.AluOpType.mult, op1=mybir.AluOpType.add)
        # Q blocks: QT[dh, s] per g
        for g in range(GRP):
            gsl = slice(g * DH, (g + 1) * DH)
            ps = psPp.tile([P, NB], f32, tag="mm")
            for pr in range(npr):
                jsl = slice(2 * pr, 2 * pr + 2)
                res_mm(ps, wq_sb[:, jsl, 0, gsl], wq_sb[:, jsl, 1, gsl],
                       xp_sb[:, jsl, sb, 0, :], xp_sb[:, jsl, sb, 1, :],
                       pr, pr == 0, pr == npr - 1)
            nc.scalar.activation(out=qt_sb[:, g, ssl], in_=ps, func=Identity,
                                 bias=bq_sb[:, g:g + 1], scale=1.0 / WSC)

    projpool.__exit__(None, None, None)
    psP.__exit__(None, None, None)

    # ================= phase A: attention =================
    psSpool = tc.tile_pool(name="psS", bufs=2, space="PSUM")
    psS = psSpool.__enter__()
    psOApool = tc.tile_pool(name="psOA", bufs=3, space="PSUM")
    psOA = psOApool.__enter__()
    psRpool = tc.tile_pool(name="psR", bufs=1, space="PSUM")
    psR = psRpool.__enter__()
    ptpool = ctx.enter_context(tc.tile_pool(name="ptp", bufs=2))
    trpool = ctx.enter_context(tc.tile_pool(name="trp", bufs=2))
    scpool = ctx.enter_context(tc.tile_pool(name="scp", bufs=2))

    npair = nt // 2
    blocks = [(g, qb) for g in range(GRP) for qb in range(nsb)]

    def emit_tree(tree, tall, tallb, acc512):
        """Finish a block's rowsum tree: combine the 4 level-1 pair-sums and
        fold [P,2,NB] -> [P,NB]."""
        nc.vector.tensor_add(tall, tree[0], tree[1])
        nc.vector.tensor_add(tallb, tree[2], tree[3])
        nc.vector.tensor_add(tall, tall, tallb)
        nc.vector.tensor_add(acc512, tall[:, 0, :], tall[:, 1, :])

    def finish_norm(pg, pqb, pacc, pps_o):
        """Rowsum matmul + normalize + fp8 hi/lo split for a finished block.
        PE: 1 fp8-DR ones-matmul; DVE: recip + mul; Pool: bcast + hi + lo."""
        pqsl = slice(pqb * NB, (pqb + 1) * NB)
        ps_r = psR.tile([1, NB], f32, tag="r")
        nc.tensor.matmul(ps_r, lhsT=ones16[:, 0:1], rhs=pacc,
                         start=True, stop=True)
        rrow = scpool.tile([1, NB], f32, tag="rrow")
        nc.vector.reciprocal(rrow, ps_r)
        rb = scpool.tile([P, NB], f32, tag="rb")
        nc.gpsimd.partition_broadcast(rb, rrow[0:1, :])
        otf = scpool.tile([P, NB], f32, tag="otf")
        nc.vector.tensor_mul(otf, pps_o, rb)
        nc.gpsimd.tensor_copy(ot8hi[:, pg, pqsl], otf)
        nc.gpsimd.tensor_sub(ot8lo[:, pg, pqsl], otf, ot8hi[:, pg, pqsl])

    def emit_block(g, qb, prev):
        """Emit one (g, q-block): scores+exp for this block interleaved on PE
        with AV of `prev`; rowsum/normalize of `prev` rides along at the end.
        Returns this block's state tuple."""
        qsl = slice(qb * NB, (qb + 1) * NB)
        pt = ptpool.tile([P, nt, NB], bf16, tag="pt")
        ps_o = psOA.tile([P, NB], f32, tag="o")
        tree = [trpool.tile([P, 2, NB], bf16, tag=f"t{i}", name=f"tree{i}")
                for i in range(4)]
        tall = trpool.tile([P, 2, NB], bf16, tag="tall")
        tallb = trpool.tile([P, 2, NB], bf16, tag="tallb")
        tall2 = trpool.tile([P, NB], bf16, tag="acc")
        if prev is not None:
            (pg, pqb, ppt, pps_o, ptree, ptall, pacc) = prev

        for p in range(npair):
            ps_s = psS.tile([P, 2, NB], f32, tag="s")
            for j in (0, 1):
                ki = 2 * p + j
                nc.tensor.matmul(
                    ps_s[:, j, :], lhsT=kt_sb[:, ki * P:(ki + 1) * P],
                    rhs=qt_sb[:, g, qsl], start=True, stop=True)
            # exp of the pair: Act p0-5 native, DVE p6 Schraudolph, p7 split
            # Act/DVE (GPSIMD cannot read PSUM, so Pool gets no exp work)
            if p < 6:
                nc.scalar.activation(
                    out=pt[:, 2 * p:2 * p + 2, :], in_=ps_s, func=Exp,
                    bias=zbias[:, 0:1], scale=SCALE)
            elif p == 6:
                nc.vector.tensor_scalar(
                    out=pt[:, 12:14, :].bitcast(i16), in0=ps_s,
                    scalar1=SCH_A, scalar2=SCH_B,
                    op0=mybir.AluOpType.mult, op1=mybir.AluOpType.add)
            else:
                nc.scalar.activation(
                    out=pt[:, 14:15, :], in_=ps_s[:, 0, :], func=Exp,
                    bias=zbias[:, 0:1], scale=SCALE)
                nc.vector.tensor_scalar(
                    out=pt[:, 15:16, :].bitcast(i16), in0=ps_s[:, 1, :],
                    scalar1=SCH_A, scalar2=SCH_B,
                    op0=mybir.AluOpType.mult, op1=mybir.AluOpType.add)
            # pairwise rowsum tree level 1 as tiles become ready
            if p % 2 == 1:
                nc.vector.tensor_add(tree[p // 2], pt[:, 2 * p - 2:2 * p, :],
                                     pt[:, 2 * p:2 * p + 2, :])
            # AV of prev interleaves with this block's scores on PE
            if prev is not None:
                nc.tensor.matmul(
                    pps_o, lhsT=v_sb[:, 2 * p, :], rhs=ppt[:, 2 * p, :],
                    start=(p == 0), stop=False)
                nc.tensor.matmul(
                    pps_o, lhsT=v_sb[:, 2 * p + 1, :], rhs=ppt[:, 2 * p + 1, :],
                    start=False, stop=(p == npair - 1))
        # fold THIS block's tree right after its last level-1 add so the DVE
        # has pacc ready well before next block's end-of-stream ones-matmul
        emit_tree(tree, tall, tallb, tall2)
        if prev is not None:
            finish_norm(pg, pqb, pacc, pps_o)
        return (g, qb, pt, ps_o, tree, tall, tall2)

    prev = None
    for g, qb in blocks:
        prev = emit_block(g, qb, prev)

    # epilogue: drain last block (AV + rowsum + normalize); its tree was
    # already folded inside emit_block
    (pg, pqb, ppt, pps_o, ptree, ptall, pacc) = prev
    for ki in range(nt):
        nc.tensor.matmul(pps_o, lhsT=v_sb[:, ki, :], rhs=ppt[:, ki, :],
                         start=(ki == 0), stop=(ki == nt - 1))
    finish_norm(pg, pqb, pacc, pps_o)
    psRpool.__exit__(None, None, None)
    psOApool.__exit__(None, None, None)
    psSpool.__exit__(None, None, None)

    # ================= phase O: out-projection =================
    psO = ctx.enter_context(tc.tile_pool(name="psO", bufs=4, space="PSUM"))
    ypool = ctx.enter_context(tc.tile_pool(name="yp", bufs=3))
    y = aps["y"]
    ngp = GRP // 2
    for st in range(nt):
        tsl = slice(st * P, (st + 1) * P)
        y_sb = ypool.tile([P, ndb, NB], bf16, tag="y")
        for db in range(ndb):
            dsl = slice(db * NB, (db + 1) * NB)
            ps_y = psO.tile([P, NB], f32, tag="y")
            for gp in range(ngp):
                jsl = slice(2 * gp, 2 * gp + 2)
                res_mm(ps_y, ot8hi[:, jsl, tsl], ot8lo[:, jsl, tsl],
                       wo_sb[:, jsl, 0, dsl], wo_sb[:, jsl, 1, dsl],
                       gp, gp == 0, gp == ngp - 1)
            if db % 2 == 0:
                nc.scalar.mul(y_sb[:, db, :], ps_y, 1.0 / (OSC * WSC))
            else:
                nc.vector.tensor_scalar_mul(y_sb[:, db, :], ps_y, 1.0 / (OSC * WSC))
            # split the final tile's store per-db so the tail is one small
            # transfer instead of a full 512KB store after the last matmul
            if st == nt - 1:
                nc.sync.dma_start(out=y[tsl, dsl], in_=y_sb[:, db, :])
        if st < nt - 1:
            nc.sync.dma_start(out=y[tsl, :], in_=y_sb)


def build_program(s=S, d=D):
    import concourse.tile as tile
    from concourse import bacc, mybir

    nc = bacc.Bacc("TRN2", target_bir_lowering=False, debug=False)
    bf16 = mybir.dt.bfloat16
    f32 = mybir.dt.float32
    e4 = mybir.dt.float8e4
    names = {
        "xp": ([d, s // NB, 2, NB], e4),
        "wq": ([d, 2, EH], e4),
        "wkv": ([d, 4, DH], e4),
        "wo": ([EH, 2, d], e4),
        "bq": ([EH], f32), "bk": ([DH], f32), "bv": ([DH], f32),
    }
    aps = {k: nc.dram_tensor(k, sh, dt, kind="ExternalInput").ap()
           for k, (sh, dt) in names.items()}
    aps["y"] = nc.dram_tensor("y", [s, d], bf16, kind="ExternalOutput").ap()
    with tile.TileContext(nc) as tc:
        with ExitStack() as ctx:
            _emit(ctx, tc, aps, s=s, d=d)
    nc.compile()
    return nc


def _res_split(v32, scale):
    """fp8 residual split of v32*scale: hi = e4(v*scale), lo = e4(v*scale-hi)."""
    e4 = ml_dtypes.float8_e4m3
    vs = v32 * np.float32(scale)
    hi = vs.astype(e4)
    lo = (vs - hi.astype(np.float32)).astype(e4)
    return hi, lo


def make_in_maps(x, Wq, bq, Wk, bk, Wv, bv, Wo, bo):
    e4 = ml_dtypes.float8_e4m3
    in_maps = []
    xsplit = []
    for b in range(B):
        xT = np.ascontiguousarray(x[b].T)  # [D, S]
        hi, lo = _res_split(xT, 1.0)
        nsb = S // NB
        xp = np.empty((D, nsb, 2, NB), dtype=e4)
        xp[:, :, 0, :] = hi.reshape(D, nsb, NB)
        xp[:, :, 1, :] = lo.reshape(D, nsb, NB)
        xsplit.append(xp)
    for b in range(B):
        xp = xsplit[b]
        for h in range(KV):
            wqh, wql = _res_split(np.ascontiguousarray(
                Wq[:, h * EH:(h + 1) * EH]), WSC)
            wq = np.empty((D, 2, EH), dtype=e4)
            wq[:, 0, :] = wqh
            wq[:, 1, :] = wql
            wkh, wkl = _res_split(np.ascontiguousarray(
                Wk[:, h * DH:(h + 1) * DH]), WSC)
            wvh, wvl = _res_split(np.ascontiguousarray(
                Wv[:, h * DH:(h + 1) * DH]), WSC)
            wkv = np.empty((D, 4, DH), dtype=e4)
            wkv[:, 0, :] = wkh
            wkv[:, 1, :] = wkl
            wkv[:, 2, :] = wvh
            wkv[:, 3, :] = wvl
            woh, wol = _res_split(np.ascontiguousarray(
                Wo[h * EH:(h + 1) * EH, :]), WSC)
            wo = np.empty((EH, 2, D), dtype=e4)
            wo[:, 0, :] = woh
            wo[:, 1, :] = wol
            in_maps.append({
                "xp": xp, "wq": wq, "wkv": wkv, "wo": wo,
                "bq": np.ascontiguousarray(bq[h * EH:(h + 1) * EH]).astype(np.float32),
                "bk": np.ascontiguousarray(bk[h * DH:(h + 1) * DH]).astype(np.float32),
                "bv": np.ascontiguousarray(bv[h * DH:(h + 1) * DH]).astype(np.float32),
            })
    return in_maps


_PROG = None


def _get_program():
    global _PROG
    if _PROG is None:
        _PROG = build_program()
    return _PROG


def run_cores(in_maps, trace=False, **kw):
    from concourse.bass_utils import run_bass_kernel_spmd
    nc = _get_program()
    return run_bass_kernel_spmd(nc, in_maps, list(range(8)), trace=trace, **kw)


def kernel(**inputs):
    x = np.asarray(inputs["x"], dtype=np.float32)
    Wq = np.asarray(inputs["Wq"], dtype=np.float32)
    bq = np.asarray(inputs["bq"], dtype=np.float32)
    Wk = np.asarray(inputs["Wk"], dtype=np.float32)
    bk = np.asarray(inputs["bk"], dtype=np.float32)
    Wv = np.asarray(inputs["Wv"], dtype=np.float32)
    bv = np.asarray(inputs["bv"], dtype=np.float32)
    Wo = np.asarray(inputs["Wo"], dtype=np.float32)
    bo = np.asarray(inputs["bo"], dtype=np.float32)

    in_maps = make_in_maps(x, Wq, bq, Wk, bk, Wv, bv, Wo, bo)
    res = run_cores(in_maps)
    out = np.empty((B, S, D), dtype=np.float32)
    for b in range(B):
        acc = res.results[b * KV]["y"].astype(np.float32)
        for h in range(1, KV):
            acc = acc + res.results[b * KV + h]["y"].astype(np.float32)
        out[b] = acc + bo[None, :]
    return out


# revision 14
# speedup vs baseline: 1.1769x; 1.0450x over previous
"""GQA attention kernel for Trainium2 (8 NeuronCores).

Problem: B=2, S=2048, D=2048, H=16 heads of DH=128, KV=4 kv heads, G=4
query heads per kv head.  Full (dense) attention, fp32 I/O.

Sharding: batch (2) x kv-head (4) = 8 cores, zero redundant FLOPs.
Each core computes, for its (batch b, kv head h):
    Q_g = x_b @ Wq[:, h,g]  (4 query heads), K = x_b @ Wk[:, h],
    V = x_b @ Wv[:, h], O_g = softmax(Q_g K^T / sqrt(DH)) V,
    y_partial = concat_g(O_g) @ Wo[h-rows, :]
Host sums the 4 kv-head partials per batch and adds bo.

On-chip strategy:
 - Projections and out-proj run as residual-fp8 DoubleRow matmuls:
   each operand is split (on host for x/W, on chip for O) into
   e4m3 hi + e4m3 lo residual; products hi.hi + hi.lo + lo.hi are kept
   (lo.lo dropped).  3 DoubleRow matmuls per 256-deep contraction pair
   = 1.5 PE cycles/row vs bf16's 2.0, at better-than-bf16 accuracy.
 - hi/lo pairs are PACKED into single DRAM tensors host-side so each
   logical load is ONE dma_start (HWDGE dispatch is ~630ns serialized;
   the old per-(sb,t) x chunking burned ~80us of dispatch).
 - Scores S^T tiles ([k, q], lhsT=KT slice, rhs=QT block) and AV
   (lhsT=V tile, rhs=exp tile) in bf16.  1/sqrt(DH) is applied inside
   the exp activation (scale operand), keeping qt/kt at unit scale.
 - exp of the 16 score k-tiles per (g, q-block): 12 tiles on the Act
   engine (native Exp, PSUM pair reads [128,1024]), 4 tiles as
   Schraudolph bit-trick exponentials on DVE (tensor_scalar
   fp32->int16 of s*A+B, bitcast to bf16), spreading exp across
   engines so the PE stays the bottleneck.
 - rowsum via DVE pairwise tree-add of exp tiles; the last tree level
   emits e4m3 partial sums so the final partition-reduce is a single
   fp8 DoubleRow ones-matmul (256 PE cycles vs 512); reciprocal on
   DVE; 1/r broadcast on Pool; normalization multiply on DVE produces
   O*16/r fp32, split into e4m3 hi/lo for the residual out-proj
   (Pool copy + Pool subtract).
 - y written bf16 (PSUM * 1/1024 scale), host sums partials in fp32.
"""

import sys

if "/opt/trn_rl_repo" not in sys.path:
    sys.path.insert(0, "/opt/trn_rl_repo")

import numpy as np
import ml_dtypes
from contextlib import ExitStack

B, S, D = 2, 2048, 2048
H, DH, GRP = 16, 128, 4
KV = H // GRP            # 4 kv heads
EH = GRP * DH            # 512 = query-head columns per kv head
SCALE = float(1.0 / np.sqrt(np.float32(DH)))
P = 128                  # partitions
NB = 512                 # matmul moving-dim block (one PSUM bank fp32)
WSC = 64.0               # weight fp8 pre-scale
OSC = 16.0               # ot fp8 pre-scale

# Schraudolph exp-approx constants (bf16 bit domain), folding in SCALE
SCH_A = float(128.0 * SCALE / np.log(2.0))
SCH_B = float((127.0 - 0.0579) * 128.0)


def _emit(ctx, tc, aps, s=S, d=D):
    import concourse.bass as bass
    from concourse import mybir

    nc = tc.nc
    bf16 = mybir.dt.bfloat16
    f32 = mybir.dt.float32
    e4 = mybir.dt.float8e4
    i16 = mybir.dt.int16
    DR = mybir.MatmulPerfMode.DoubleRow
    Exp = mybir.ActivationFunctionType.Exp
    Identity = mybir.ActivationFunctionType.Identity

    nt = s // P           # 128-tiles along s
    nd = d // P           # 128-tiles along d (contraction)
    npr = nd // 2         # 256-pairs along d
    nsb = s // NB         # 512-blocks along s
    ndb = d // NB         # 512-blocks along d (out columns)

    persist = ctx.enter_context(tc.tile_pool(name="persist", bufs=1))

    # ---- persistent tiles ----
    wo_sb = persist.tile([P, GRP, 2, d], e4)      # [.,g,hi/lo,d]
    qt_sb = persist.tile([P, GRP, s], bf16)
    kt_sb = persist.tile([P, s], bf16)
    v_sb = persist.tile([P, nt, DH], bf16)
    ot8hi = persist.tile([P, GRP, s], e4)
    ot8lo = persist.tile([P, GRP, s], e4)
    bq_sb = persist.tile([P, GRP], f32)
    bk_sb = persist.tile([P, 1], f32)
    bvb_sb = persist.tile([P, DH], f32)
    zbias = persist.tile([P, 1], f32)

    nc.vector.memset(zbias, 0.0)

    # ================= phase P: projections =================
    projpool = tc.tile_pool(name="projp", bufs=1)
    projp = projpool.__enter__()
    psP = tc.tile_pool(name="psP", bufs=2, space="PSUM")
    psPp = psP.__enter__()

    xp_sb = projp.tile([P, nd, nsb, 2, NB], e4)   # [.,t,sb,hi/lo,n]
    wq_sb = projp.tile([P, GRP, nd, 2, DH], e4)   # [.,g,t,hi/lo,dh]
    wkv_sb = projp.tile([P, nd, 4, DH], e4)       # [.,t,khi/klo/vhi/vlo,dh]

    # loads, ordered for earliest PE start (HWDGE dispatch is serial):
    # wkv first (gates K/V of sb0), then x sb0 split in two t-halves
    # (prs start as soon as the first half lands), biases, wq (only
    # needed ~10us in, when Q blocks start), x sb1-3, wo last.
    xp_r = aps["xp"].rearrange("(t p) b h n -> p t b h n", p=P)
    wkv_r = aps["wkv"].rearrange("(t p) c e -> p t c e", p=P)
    wq_r = aps["wq"].rearrange("g (t p) h e -> p g t h e", p=P)
    nc.sync.dma_start(out=wkv_sb, in_=wkv_r)
    nc.sync.dma_start(out=xp_sb[:, 0:4, 0, :, :], in_=xp_r[:, 0:4, 0, :, :])
    nc.sync.dma_start(out=bk_sb, in_=aps["bk"].rearrange("(p o) -> p o", o=1))
    nc.sync.dma_start(out=xp_sb[:, 4:8, 0, :, :], in_=xp_r[:, 4:8, 0, :, :])
    nc.sync.dma_start(out=xp_sb[:, 8:12, 0, :, :], in_=xp_r[:, 8:12, 0, :, :])
    nc.sync.dma_start(out=xp_sb[:, 12:16, 0, :, :], in_=xp_r[:, 12:16, 0, :, :])
    bv = aps["bv"]
    bv_bcast = bass.AP(tensor=bv.tensor, offset=bv.offset,
                       ap=[[0, P]] + list(bv.ap))
    nc.sync.dma_start(out=bvb_sb, in_=bv_bcast)
    nc.sync.dma_start(out=bq_sb, in_=aps["bq"].rearrange("(g p) -> p g", p=P))
    for g in range(GRP):
        nc.sync.dma_start(out=wq_sb[:, g], in_=wq_r[:, g])
    for sb in range(1, nsb):
        nc.sync.dma_start(out=xp_sb[:, :, sb, :, :], in_=xp_r[:, :, sb, :, :])
    nc.sync.dma_start(out=wo_sb, in_=aps["wo"].rearrange("(g p) h d -> p g h d", p=P))

    def res_mm(ps, w_hi, w_lo, x_hi, x_lo, pr, first, last):
        """3 DoubleRow matmuls for one 256-deep pair: hi.hi + hi.lo + lo.hi.
        w_* are lhsT [P, 2, m] APs, x_* are rhs [P, 2, n] APs."""
        nc.tensor.matmul(ps, lhsT=w_hi, rhs=x_hi, start=first, stop=False,
                         perf_mode=DR)
        nc.tensor.matmul(ps, lhsT=w_lo, rhs=x_hi, start=False, stop=False,
                         perf_mode=DR)
        nc.tensor.matmul(ps, lhsT=w_hi, rhs=x_lo, start=False, stop=last,
                         perf_mode=DR)

    # sb-major so each s-block's K/V/Q runs as soon as its x columns land;
    # K first within the sb (kt completeness gates the attention phase).
    for sb in range(nsb):
        ssl = slice(sb * NB, (sb + 1) * NB)
        # K block: KT[dh, s]
        ps = psPp.tile([P, NB], f32, tag="mm")
        for pr in range(npr):
            jsl = slice(2 * pr, 2 * pr + 2)
            res_mm(ps, wkv_sb[:, jsl, 0, :], wkv_sb[:, jsl, 1, :],
                   xp_sb[:, jsl, sb, 0, :], xp_sb[:, jsl, sb, 1, :],
                   pr, pr == 0, pr == npr - 1)
        nc.scalar.activation(out=kt_sb[:, ssl], in_=ps, func=Identity,
                             bias=bk_sb[:, 0:1], scale=1.0 / WSC)
        # V tiles: V[k, dh] natural (x stationary, wv moving)
        for st in range(4 * sb, 4 * sb + 4):
            csl = slice((st % 4) * P, (st % 4 + 1) * P)
            ps = psPp.tile([P, NB], f32, tag="mm")
            for pr in range(npr):
                jsl = slice(2 * pr, 2 * pr + 2)
                res_mm(ps[:, 0:DH],
                       xp_sb[:, jsl, sb, 0, csl], xp_sb[:, jsl, sb, 1, csl],
                       wkv_sb[:, jsl, 2, :], wkv_sb[:, jsl, 3, :],
                       pr, pr == 0, pr == npr - 1)
            nc.vector.scalar_tensor_tensor(
                out=v_sb[:, st, :], in0=ps[:, 0:DH], scalar=OSC / WSC,
                in1=bvb_sb, op0=mybir.AluOpType.mult, op1=mybir.AluOpType.add)
        # Q blocks: QT[dh, s] per g
        for g in range(GRP):
            ps = psPp.tile([P, NB], f32, tag="mm")
            for pr in range(npr):
                jsl = slice(2 * pr, 2 * pr + 2)
                res_mm(ps, wq_sb[:, g, jsl, 0, :], wq_sb[:, g, jsl, 1, :],
                       xp_sb[:, jsl, sb, 0, :], xp_sb[:, jsl, sb, 1, :],
                       pr, pr == 0, pr == npr - 1)
            nc.scalar.activation(out=qt_sb[:, g, ssl], in_=ps, func=Identity,
                                 bias=bq_sb[:, g:g + 1], scale=1.0 / WSC)

    projpool.__exit__(None, None, None)
    psP.__exit__(None, None, None)

    # ================= phase A: attention =================
    psS = ctx.enter_context(tc.tile_pool(name="psS", bufs=2, space="PSUM"))
    psOA = ctx.enter_context(tc.tile_pool(name="psOA", bufs=2, space="PSUM"))
    ptpool = ctx.enter_context(tc.tile_pool(name="ptp", bufs=2))
    trpool = ctx.enter_context(tc.tile_pool(name="trp", bufs=2))
    scpool = ctx.enter_context(tc.tile_pool(name="scp", bufs=2))

    npair = nt // 2
    blocks = [(g, qb) for qb in range(nsb) for g in range(GRP)]

    def emit_tree(tree, tall, tallb, acc512):
        """Finish a block's rowsum tree: combine the 4 level-1 pair-sums and
        fold [P,2,NB] -> [P,NB].  L2 on DVE; L3 + fold on Pool (slow but it
        has slack), keeping DVE under the PE block budget."""
        nc.vector.tensor_add(tall, tree[0], tree[1])
        nc.vector.tensor_add(tallb, tree[2], tree[3])
        nc.vector.tensor_add(tall, tall, tallb)
        nc.vector.tensor_add(acc512, tall[:, 0, :], tall[:, 1, :])

    def finish_norm(pg, pqb, pacc, pps_o):
        """Rowsum all-reduce + normalize + fp8 hi/lo split for a finished
        block.  Pool: partition all-reduce + hi + lo; DVE: recip + mul.
        (v_sb carries the x16 out-scale, so otf = O*16/r directly.)"""
        import concourse.bass_isa as bass_isa
        pqsl = slice(pqb * NB, (pqb + 1) * NB)
        rall = scpool.tile([P, NB], f32, tag="rall")
        nc.gpsimd.partition_all_reduce(rall, pacc, channels=P,
                                       reduce_op=bass_isa.ReduceOp.add)
        rb = scpool.tile([P, NB], f32, tag="rb")
        nc.vector.reciprocal(rb, rall)
        otf = scpool.tile([P, NB], f32, tag="otf")
        nc.vector.tensor_mul(otf, pps_o, rb)
        nc.gpsimd.tensor_copy(ot8hi[:, pg, pqsl], otf)
        nc.gpsimd.tensor_sub(ot8lo[:, pg, pqsl], otf, ot8hi[:, pg, pqsl])

    def emit_block(g, qb, prev):
        """Emit one (g, q-block): scores+exp for this block interleaved on PE
        with AV of `prev`; rowsum/normalize of `prev` rides along at the end.
        Returns this block's state tuple."""
        qsl = slice(qb * NB, (qb + 1) * NB)
        pt = ptpool.tile([P, nt, NB], bf16, tag="pt")
        ps_o = psOA.tile([P, NB], f32, tag="o")
        tree = [trpool.tile([P, 2, NB], bf16, tag=f"t{i}", name=f"tree{i}")
                for i in range(4)]
        tall = trpool.tile([P, 2, NB], bf16, tag="tall")
        tallb = trpool.tile([P, 2, NB], bf16, tag="tallb")
        tall2 = trpool.tile([P, NB], bf16, tag="acc")
        if prev is not None:
            (pg, pqb, ppt, pps_o, ptree, ptall, pacc) = prev

        for p in range(npair):
            ps_s = psS.tile([P, 2, NB], f32, tag="s")
            for j in (0, 1):
                ki = 2 * p + j
                nc.tensor.matmul(
                    ps_s[:, j, :], lhsT=kt_sb[:, ki * P:(ki + 1) * P],
                    rhs=qt_sb[:, g, qsl], start=True, stop=True)
            # exp of the pair: all on Act (out-proj interleave lifts the
            # per-block PE budget above Act's 8-pair cost, so no Schraudolph
            # approximation is needed any more)
            nc.scalar.activation(
                out=pt[:, 2 * p:2 * p + 2, :], in_=ps_s, func=Exp,
                bias=zbias[:, 0:1], scale=SCALE)
            # pairwise rowsum tree level 1 as tiles become ready
            if p % 2 == 1:
                nc.vector.tensor_add(tree[p // 2], pt[:, 2 * p - 2:2 * p, :],
                                     pt[:, 2 * p:2 * p + 2, :])
            # AV of prev interleaves with this block's scores on PE
            if prev is not None:
                nc.tensor.matmul(
                    pps_o, lhsT=v_sb[:, 2 * p, :], rhs=ppt[:, 2 * p, :],
                    start=(p == 0), stop=False)
                nc.tensor.matmul(
                    pps_o, lhsT=v_sb[:, 2 * p + 1, :], rhs=ppt[:, 2 * p + 1, :],
                    start=False, stop=(p == npair - 1))
        # fold THIS block's tree right after its last level-1 add so the DVE
        # has pacc ready well before next block's end-of-stream ones-matmul
        emit_tree(tree, tall, tallb, tall2)
        if prev is not None:
            finish_norm(pg, pqb, pacc, pps_o)
        return (g, qb, pt, ps_o, tree, tall, tall2)

    # out-projection tiles are interleaved into the attention stream: tile
    # st (qb = st//4) is emitted once all four g-blocks of its qb have been
    # normalized (safe after attention block index 4*qb+5), filling PE
    # bubbles left by the exp/rowsum pipeline.
    psO = ctx.enter_context(tc.tile_pool(name="psO", bufs=2, space="PSUM"))
    ypool = ctx.enter_context(tc.tile_pool(name="yp", bufs=3))
    y = aps["y"]
    ngp = GRP // 2

    def emit_outproj_tile(st):
        tsl = slice(st * P, (st + 1) * P)
        y_sb = ypool.tile([P, ndb, NB], bf16, tag="y")
        for db in range(ndb):
            dsl = slice(db * NB, (db + 1) * NB)
            ps_y = psO.tile([P, NB], f32, tag="y")
            for gp in range(ngp):
                jsl = slice(2 * gp, 2 * gp + 2)
                res_mm(ps_y, ot8hi[:, jsl, tsl], ot8lo[:, jsl, tsl],
                       wo_sb[:, jsl, 0, dsl], wo_sb[:, jsl, 1, dsl],
                       gp, gp == 0, gp == ngp - 1)
            if db == 0:
                nc.scalar.mul(y_sb[:, db, :], ps_y, 1.0 / (OSC * WSC))
            else:
                nc.vector.tensor_scalar_mul(y_sb[:, db, :], ps_y, 1.0 / (OSC * WSC))
            # split the final tile's store per-db so the tail is one small
            # transfer instead of a full 512KB store after the last matmul
            if st == nt - 1:
                nc.sync.dma_start(out=y[tsl, dsl], in_=y_sb[:, db, :])
        if st < nt - 1:
            nc.sync.dma_start(out=y[tsl, :], in_=y_sb)

    prev = None
    for i, (g, qb) in enumerate(blocks):
        prev = emit_block(g, qb, prev)
        if i >= 5:
            emit_outproj_tile(i - 5)
    # st10/st11 (qb2) are ready after the last block; st12-14 (qb3) need
    # norms of blocks 12-14, which land during blocks 14/15/epilogue.  Only
    # st15 must wait for the final block's normalization chain.
    emit_outproj_tile(11)

    # epilogue: drain last block (AV + rowsum + normalize); its tree was
    # already folded inside emit_block
    (pg, pqb, ppt, pps_o, ptree, ptall, pacc) = prev
    for ki in range(nt):
        nc.tensor.matmul(pps_o, lhsT=v_sb[:, ki, :], rhs=ppt[:, ki, :],
                         start=(ki == 0), stop=(ki == nt - 1))
    emit_outproj_tile(12)
    emit_outproj_tile(13)
    finish_norm(pg, pqb, pacc, pps_o)
    emit_outproj_tile(14)
    emit_outproj_tile(15)


def build_program(s=S, d=D):
    import concourse.tile as tile
    from concourse import bacc, mybir

    nc = bacc.Bacc("TRN2", target_bir_lowering=False, debug=False)
    bf16 = mybir.dt.bfloat16
    f32 = mybir.dt.float32
    e4 = mybir.dt.float8e4
    names = {
        "xp": ([d, s // NB, 2, NB], e4),
        "wq": ([GRP, d, 2, DH], e4),
        "wkv": ([d, 4, DH], e4),
        "wo": ([EH, 2, d], e4),
        "bq": ([EH], f32), "bk": ([DH], f32), "bv": ([DH], f32),
    }
    aps = {k: nc.dram_tensor(k, sh, dt, kind="ExternalInput").ap()
           for k, (sh, dt) in names.items()}
    aps["y"] = nc.dram_tensor("y", [s, d], bf16, kind="ExternalOutput").ap()
    with tile.TileContext(nc) as tc:
        with ExitStack() as ctx:
            _emit(ctx, tc, aps, s=s, d=d)
    nc.compile()
    return nc


def _res_split(v32, scale):
    """fp8 residual split of v32*scale: hi = e4(v*scale), lo = e4(v*scale-hi)."""
    e4 = ml_dtypes.float8_e4m3
    vs = v32 * np.float32(scale)
    hi = vs.astype(e4)
    lo = (vs - hi.astype(np.float32)).astype(e4)
    return hi, lo


def make_in_maps(x, Wq, bq, Wk, bk, Wv, bv, Wo, bo):
    e4 = ml_dtypes.float8_e4m3
    in_maps = []
    xsplit = []
    for b in range(B):
        xT = np.ascontiguousarray(x[b].T)  # [D, S]
        hi, lo = _res_split(xT, 1.0)
        nsb = S // NB
        xp = np.empty((D, nsb, 2, NB), dtype=e4)
        xp[:, :, 0, :] = hi.reshape(D, nsb, NB)
        xp[:, :, 1, :] = lo.reshape(D, nsb, NB)
        xsplit.append(xp)
    for b in range(B):
        xp = xsplit[b]
        for h in range(KV):
            wqh, wql = _res_split(np.ascontiguousarray(
                Wq[:, h * EH:(h + 1) * EH]), WSC)
            wq = np.empty((GRP, D, 2, DH), dtype=e4)
            for g in range(GRP):
                wq[g, :, 0, :] = wqh[:, g * DH:(g + 1) * DH]
                wq[g, :, 1, :] = wql[:, g * DH:(g + 1) * DH]
            wkh, wkl = _res_split(np.ascontiguousarray(
                Wk[:, h * DH:(h + 1) * DH]), WSC)
            wvh, wvl = _res_split(np.ascontiguousarray(
                Wv[:, h * DH:(h + 1) * DH]), WSC)
            wkv = np.empty((D, 4, DH), dtype=e4)
            wkv[:, 0, :] = wkh
            wkv[:, 1, :] = wkl
            wkv[:, 2, :] = wvh
            wkv[:, 3, :] = wvl
            woh, wol = _res_split(np.ascontiguousarray(
                Wo[h * EH:(h + 1) * EH, :]), WSC)
            wo = np.empty((EH, 2, D), dtype=e4)
            wo[:, 0, :] = woh
            wo[:, 1, :] = wol
            in_maps.append({
                "xp": xp, "wq": wq, "wkv": wkv, "wo": wo,
                "bq": np.ascontiguousarray(bq[h * EH:(h + 1) * EH]).astype(np.float32),
                "bk": np.ascontiguousarray(bk[h * DH:(h + 1) * DH]).astype(np.float32),
                "bv": np.ascontiguousarray(bv[h * DH:(h + 1) * DH]).astype(np.float32),
            })
    return in_maps


_PROG = None


def _get_program():
    global _PROG
    if _PROG is None:
        _PROG = build_program()
    return _PROG


def run_cores(in_maps, trace=False, **kw):
    from concourse.bass_utils import run_bass_kernel_spmd
    nc = _get_program()
    return run_bass_kernel_spmd(nc, in_maps, list(range(8)), trace=trace, **kw)


def kernel(**inputs):
    x = np.asarray(inputs["x"], dtype=np.float32)
    Wq = np.asarray(inputs["Wq"], dtype=np.float32)
    bq = np.asarray(inputs["bq"], dtype=np.float32)
    Wk = np.asarray(inputs["Wk"], dtype=np.float32)
    bk = np.asarray(inputs["bk"], dtype=np.float32)
    Wv = np.asarray(inputs["Wv"], dtype=np.float32)
    bv = np.asarray(inputs["bv"], dtype=np.float32)
    Wo = np.asarray(inputs["Wo"], dtype=np.float32)
    bo = np.asarray(inputs["bo"], dtype=np.float32)

    in_maps = make_in_maps(x, Wq, bq, Wk, bk, Wv, bv, Wo, bo)
    res = run_cores(in_maps)
    out = np.empty((B, S, D), dtype=np.float32)
    for b in range(B):
        acc = res.results[b * KV]["y"].astype(np.float32)
        for h in range(1, KV):
            acc = acc + res.results[b * KV + h]["y"].astype(np.float32)
        out[b] = acc + bo[None, :]
    return out


# revision 23
# speedup vs baseline: 1.1911x; 1.0121x over previous
"""GQA attention kernel for Trainium2 (8 NeuronCores).

Problem: B=2, S=2048, D=2048, H=16 heads of DH=128, KV=4 kv heads, G=4
query heads per kv head.  Full (dense) attention, fp32 I/O.

Sharding: batch (2) x kv-head (4) = 8 cores, zero redundant FLOPs.
Each core computes, for its (batch b, kv head h):
    Q_g = x_b @ Wq[:, h,g]  (4 query heads), K = x_b @ Wk[:, h],
    V = x_b @ Wv[:, h], O_g = softmax(Q_g K^T / sqrt(DH)) V,
    y_partial = concat_g(O_g) @ Wo[h-rows, :]
Host sums the 4 kv-head partials per batch and adds bo.

On-chip strategy:
 - Projections and out-proj run as residual-fp8 DoubleRow matmuls:
   each operand is split (on host for x/W, on chip for O) into
   e4m3 hi + e4m3 lo residual; products hi.hi + hi.lo + lo.hi are kept
   (lo.lo dropped).  3 DoubleRow matmuls per 256-deep contraction pair
   = 1.5 PE cycles/row vs bf16's 2.0, at better-than-bf16 accuracy.
 - hi/lo pairs are PACKED into single DRAM tensors host-side so each
   logical load is ONE dma_start (HWDGE dispatch is ~630ns serialized;
   the old per-(sb,t) x chunking burned ~80us of dispatch).
 - Scores S^T tiles ([k, q], lhsT=KT slice, rhs=QT block) and AV
   (lhsT=V tile, rhs=exp tile) in bf16.  1/sqrt(DH) is applied inside
   the exp activation (scale operand), keeping qt/kt at unit scale.
 - exp of the 16 score k-tiles per (g, q-block): 12 tiles on the Act
   engine (native Exp, PSUM pair reads [128,1024]), 4 tiles as
   Schraudolph bit-trick exponentials on DVE (tensor_scalar
   fp32->int16 of s*A+B, bitcast to bf16), spreading exp across
   engines so the PE stays the bottleneck.
 - rowsum via DVE pairwise tree-add of exp tiles; the last tree level
   emits e4m3 partial sums so the final partition-reduce is a single
   fp8 DoubleRow ones-matmul (256 PE cycles vs 512); reciprocal on
   DVE; 1/r broadcast on Pool; normalization multiply on DVE produces
   O*16/r fp32, split into e4m3 hi/lo for the residual out-proj
   (Pool copy + Pool subtract).
 - y written bf16 (PSUM * 1/1024 scale), host sums partials in fp32.
"""

import sys

if "/opt/trn_rl_repo" not in sys.path:
    sys.path.insert(0, "/opt/trn_rl_repo")

import numpy as np
import ml_dtypes
from contextlib import ExitStack

B, S, D = 2, 2048, 2048
H, DH, GRP = 16, 128, 4
KV = H // GRP            # 4 kv heads
EH = GRP * DH            # 512 = query-head columns per kv head
SCALE = float(1.0 / np.sqrt(np.float32(DH)))
P = 128                  # partitions
NB = 512                 # matmul moving-dim block (one PSUM bank fp32)
WSC = 64.0               # weight fp8 pre-scale
OSC = 16.0               # ot fp8 pre-scale

# Schraudolph exp-approx constants (bf16 bit domain), folding in SCALE and
# the 2^-5 range pre-scale applied to all exp tiles
SCH_A = float(128.0 * SCALE / np.log(2.0))
SCH_B = float((127.0 - 5.0 - 0.0579) * 128.0)


def _emit(ctx, tc, aps, s=S, d=D):
    import concourse.bass as bass
    from concourse import mybir

    nc = tc.nc
    bf16 = mybir.dt.bfloat16
    f32 = mybir.dt.float32
    e4 = mybir.dt.float8e4
    i16 = mybir.dt.int16
    DR = mybir.MatmulPerfMode.DoubleRow
    Exp = mybir.ActivationFunctionType.Exp
    Identity = mybir.ActivationFunctionType.Identity

    nt = s // P           # 128-tiles along s
    nd = d // P           # 128-tiles along d (contraction)
    npr = nd // 2         # 256-pairs along d
    nsb = s // NB         # 512-blocks along s
    ndb = d // NB         # 512-blocks along d (out columns)

    persist = ctx.enter_context(tc.tile_pool(name="persist", bufs=1))

    # ---- persistent tiles ----
    wo_sb = persist.tile([P, GRP, 2, d], e4)      # [.,g,hi/lo,d]
    qt_sb = persist.tile([P, GRP, s], bf16)
    kt_sb = persist.tile([P, s], bf16)
    v_sb = persist.tile([P, nt, DH], bf16)
    v8hi = persist.tile([P, 4, DH], e4)
    v8lo = persist.tile([P, 4, DH], e4)
    ot8hi = persist.tile([P, GRP, s], e4)
    ot8lo = persist.tile([P, GRP, s], e4)
    bq_sb = persist.tile([P, GRP], f32)
    bk_sb = persist.tile([P, 1], f32)
    bvb_sb = persist.tile([P, DH], f32)
    zbias = persist.tile([P, 1], f32)

    # exp bias: scale all exp by 2^-5 so the e4m3 tiles stay in range
    # (max score*SCALE ~ 6.9 -> exp*2^-5 <= 32.1 << 448); cancels in O/r
    nc.vector.memset(zbias, float(-5.0 * np.log(2.0)))

    # ================= phase P: projections =================
    projpool = tc.tile_pool(name="projp", bufs=1)
    projp = projpool.__enter__()
    psP = tc.tile_pool(name="psP", bufs=2, space="PSUM")
    psPp = psP.__enter__()

    xp_sb = projp.tile([P, nd, nsb, 2, NB], e4)   # [.,t,sb,hi/lo,n]
    wq_sb = projp.tile([P, GRP, nd, 2, DH], e4)   # [.,g,t,hi/lo,dh]
    wkv_sb = projp.tile([P, nd, 4, DH], e4)       # [.,t,khi/klo/vhi/vlo,dh]

    # loads, ordered for earliest PE start (HWDGE dispatch is serial):
    # wkv first (gates K/V of sb0), then x sb0 split in two t-halves
    # (prs start as soon as the first half lands), biases, wq (only
    # needed ~10us in, when Q blocks start), x sb1-3, wo last.
    xp_r = aps["xp"].rearrange("(t p) b h n -> p t b h n", p=P)
    wkv_r = aps["wkv"].rearrange("(t p) c e -> p t c e", p=P)
    wq_r = aps["wq"].rearrange("g (t p) h e -> p g t h e", p=P)
    nc.sync.dma_start(out=wkv_sb, in_=wkv_r)
    nc.sync.dma_start(out=xp_sb[:, 0:nd // 2, 0, :, :],
                      in_=xp_r[:, 0:nd // 2, 0, :, :])
    nc.sync.dma_start(out=bk_sb, in_=aps["bk"].rearrange("(p o) -> p o", o=1))
    nc.sync.dma_start(out=xp_sb[:, nd // 2:, 0, :, :],
                      in_=xp_r[:, nd // 2:, 0, :, :])
    bv = aps["bv"]
    bv_bcast = bass.AP(tensor=bv.tensor, offset=bv.offset,
                       ap=[[0, P]] + list(bv.ap))
    nc.sync.dma_start(out=bvb_sb, in_=bv_bcast)
    nc.sync.dma_start(out=bq_sb, in_=aps["bq"].rearrange("(g p) -> p g", p=P))
    for g in range(GRP):
        nc.sync.dma_start(out=wq_sb[:, g], in_=wq_r[:, g])
    for sb in range(1, nsb):
        nc.sync.dma_start(out=xp_sb[:, :, sb, :, :], in_=xp_r[:, :, sb, :, :])
    nc.sync.dma_start(out=wo_sb, in_=aps["wo"].rearrange("(g p) h d -> p g h d", p=P))

    def res_mm(ps, w_hi, w_lo, x_hi, x_lo, pr, first, last):
        """3 DoubleRow matmuls for one 256-deep pair: hi.hi + hi.lo + lo.hi.
        w_* are lhsT [P, 2, m] APs, x_* are rhs [P, 2, n] APs."""
        nc.tensor.matmul(ps, lhsT=w_hi, rhs=x_hi, start=first, stop=False,
                         perf_mode=DR)
        nc.tensor.matmul(ps, lhsT=w_lo, rhs=x_hi, start=False, stop=False,
                         perf_mode=DR)
        nc.tensor.matmul(ps, lhsT=w_hi, rhs=x_lo, start=False, stop=last,
                         perf_mode=DR)

    # sb-major so each s-block's K/V/Q runs as soon as its x columns land;
    # K first within the sb (kt completeness gates the attention phase).
    for sb in range(nsb):
        ssl = slice(sb * NB, (sb + 1) * NB)
        # K block: KT[dh, s]
        ps = psPp.tile([P, NB], f32, tag="mm")
        for pr in range(npr):
            jsl = slice(2 * pr, 2 * pr + 2)
            res_mm(ps, wkv_sb[:, jsl, 0, :], wkv_sb[:, jsl, 1, :],
                   xp_sb[:, jsl, sb, 0, :], xp_sb[:, jsl, sb, 1, :],
                   pr, pr == 0, pr == npr - 1)
        nc.scalar.activation(out=kt_sb[:, ssl], in_=ps, func=Identity,
                             bias=bk_sb[:, 0:1], scale=1.0 / WSC)
        # V tiles: V[k, dh] natural (x stationary, wv moving)
        for st in range(4 * sb, 4 * sb + 4):
            csl = slice((st % 4) * P, (st % 4 + 1) * P)
            ps = psPp.tile([P, NB], f32, tag="mm")
            for pr in range(npr):
                jsl = slice(2 * pr, 2 * pr + 2)
                res_mm(ps[:, 0:DH],
                       xp_sb[:, jsl, sb, 0, csl], xp_sb[:, jsl, sb, 1, csl],
                       wkv_sb[:, jsl, 2, :], wkv_sb[:, jsl, 3, :],
                       pr, pr == 0, pr == npr - 1)
            nc.vector.scalar_tensor_tensor(
                out=v_sb[:, st, :], in0=ps[:, 0:DH], scalar=OSC / WSC,
                in1=bvb_sb, op0=mybir.AluOpType.mult, op1=mybir.AluOpType.add)
            if st < 4:
                # e4m3 residual of (16*V) for the fp8 DoubleRow AV tiles
                nc.gpsimd.tensor_copy(v8hi[:, st, :], v_sb[:, st, :])
                nc.gpsimd.tensor_sub(v8lo[:, st, :], v_sb[:, st, :],
                                     v8hi[:, st, :])
        # Q blocks: QT[dh, s] per g
        for g in range(GRP):
            ps = psPp.tile([P, NB], f32, tag="mm")
            for pr in range(npr):
                jsl = slice(2 * pr, 2 * pr + 2)
                res_mm(ps, wq_sb[:, g, jsl, 0, :], wq_sb[:, g, jsl, 1, :],
                       xp_sb[:, jsl, sb, 0, :], xp_sb[:, jsl, sb, 1, :],
                       pr, pr == 0, pr == npr - 1)
            nc.scalar.activation(out=qt_sb[:, g, ssl], in_=ps, func=Identity,
                                 bias=bq_sb[:, g:g + 1], scale=1.0 / WSC)

    projpool.__exit__(None, None, None)
    psP.__exit__(None, None, None)

    # ================= phase A: attention =================
    psS = ctx.enter_context(tc.tile_pool(name="psS", bufs=2, space="PSUM"))
    psOA = ctx.enter_context(tc.tile_pool(name="psOA", bufs=2, space="PSUM"))
    ptpool = ctx.enter_context(tc.tile_pool(name="ptp", bufs=2))
    trpool = ctx.enter_context(tc.tile_pool(name="trp", bufs=2))
    scpool = ctx.enter_context(tc.tile_pool(name="scp", bufs=2))

    npair = nt // 2
    blocks = [(g, qb) for qb in range(nsb) for g in range(GRP)]

    def emit_tree(tree, tall, tallb, acc512):
        """Finish a block's rowsum tree: combine the 4 level-1 pair-sums and
        fold [P,2,NB] -> [P,NB].  L2 on DVE; L3 + fold on Pool (slow but it
        has slack), keeping DVE under the PE block budget."""
        nc.vector.tensor_add(tall, tree[0], tree[1])
        nc.vector.tensor_add(tallb, tree[2], tree[3])
        nc.vector.tensor_add(tall, tall, tallb)
        nc.vector.tensor_add(acc512, tall[:, 0, :], tall[:, 1, :])

    def finish_norm(pg, pqb, pacc, pps_o):
        """Rowsum all-reduce + normalize + fp8 hi/lo split for a finished
        block.  Pool: partition all-reduce + hi + lo; DVE: recip + mul.
        (v_sb carries the x16 out-scale, so otf = O*16/r directly.)"""
        import concourse.bass_isa as bass_isa
        pqsl = slice(pqb * NB, (pqb + 1) * NB)
        rall = scpool.tile([P, NB], f32, tag="rall")
        nc.gpsimd.partition_all_reduce(rall, pacc, channels=P,
                                       reduce_op=bass_isa.ReduceOp.add)
        rb = scpool.tile([P, NB], f32, tag="rb")
        nc.vector.reciprocal(rb, rall)
        otf = scpool.tile([P, NB], f32, tag="otf")
        nc.vector.tensor_mul(otf, pps_o, rb)
        nc.gpsimd.tensor_copy(ot8hi[:, pg, pqsl], otf)
        nc.gpsimd.tensor_sub(ot8lo[:, pg, pqsl], otf, ot8hi[:, pg, pqsl])

    def emit_block(g, qb, prev):
        """Emit one (g, q-block): scores+exp for this block interleaved on PE
        with AV of `prev`; rowsum/normalize of `prev` rides along at the end.
        Returns this block's state tuple."""
        qsl = slice(qb * NB, (qb + 1) * NB)
        pt8 = ptpool.tile([P, 4, NB], e4, tag="pt8")
        pt = ptpool.tile([P, nt - 4, NB], bf16, tag="pt")
        ps_o = psOA.tile([P, NB], f32, tag="o")
        tree = [trpool.tile([P, 2, NB], bf16, tag=f"t{i}", name=f"tree{i}")
                for i in range(4)]
        tall = trpool.tile([P, 2, NB], bf16, tag="tall")
        tallb = trpool.tile([P, 2, NB], bf16, tag="tallb")
        tall2 = trpool.tile([P, NB], bf16, tag="acc")
        if prev is not None:
            (pg, pqb, ppt8, ppt, pps_o, ptree, ptall, pacc) = prev

        for p in range(npair):
            ps_s = psS.tile([P, 2, NB], f32, tag="s")
            for j in (0, 1):
                ki = 2 * p + j
                nc.tensor.matmul(
                    ps_s[:, j, :], lhsT=kt_sb[:, ki * P:(ki + 1) * P],
                    rhs=qt_sb[:, g, qsl], start=True, stop=True)
            # exp of the pair: all on Act (out-proj interleave lifts the
            # per-block PE budget above Act's 8-pair cost).  k-tiles 0-7
            # emit e4m3 (feeding fp8 DoubleRow AV), tiles 8-15 bf16.
            if p < 2:
                nc.scalar.activation(
                    out=pt8[:, 2 * p:2 * p + 2, :], in_=ps_s, func=Exp,
                    bias=zbias[:, 0:1], scale=SCALE)
            else:
                nc.scalar.activation(
                    out=pt[:, 2 * p - 4:2 * p - 2, :], in_=ps_s, func=Exp,
                    bias=zbias[:, 0:1], scale=SCALE)
            # pairwise rowsum tree level 1: the e4 quartet on Pool, bf16
            # pairs on DVE
            if p == 1:
                nc.vector.tensor_add(tree[0], pt8[:, 0:2, :], pt8[:, 2:4, :])
            elif p == 3:
                nc.vector.tensor_add(tree[1], pt[:, 0:2, :], pt[:, 2:4, :])
            elif p == 5:
                nc.vector.tensor_add(tree[2], pt[:, 4:6, :], pt[:, 6:8, :])
            elif p == 7:
                nc.vector.tensor_add(tree[3], pt[:, 8:10, :], pt[:, 10:12, :])
            # AV of prev interleaves with this block's scores on PE:
            # fp8 DoubleRow pairs for k-tiles 0-7, bf16 for 8-15
            if prev is not None:
                if p < 2:
                    psl = slice(2 * p, 2 * p + 2)
                    nc.tensor.matmul(
                        pps_o, lhsT=v8hi[:, psl, :], rhs=ppt8[:, psl, :],
                        start=(p == 0), stop=False, perf_mode=DR)
                    nc.tensor.matmul(
                        pps_o, lhsT=v8lo[:, psl, :], rhs=ppt8[:, psl, :],
                        start=False, stop=False, perf_mode=DR)
                else:
                    nc.tensor.matmul(
                        pps_o, lhsT=v_sb[:, 2 * p, :], rhs=ppt[:, 2 * p - 4, :],
                        start=False, stop=False)
                    nc.tensor.matmul(
                        pps_o, lhsT=v_sb[:, 2 * p + 1, :],
                        rhs=ppt[:, 2 * p - 3, :],
                        start=False, stop=(p == npair - 1))
        # fold THIS block's tree right after its last level-1 add so the DVE
        # has pacc ready well before next block's end-of-stream reduce
        emit_tree(tree, tall, tallb, tall2)
        if prev is not None:
            finish_norm(pg, pqb, pacc, pps_o)
        return (g, qb, pt8, pt, ps_o, tree, tall, tall2)

    # out-projection tiles are interleaved into the attention stream: tile
    # st (qb = st//4) is emitted once all four g-blocks of its qb have been
    # normalized (safe after attention block index 4*qb+5), filling PE
    # bubbles left by the exp/rowsum pipeline.
    psO = ctx.enter_context(tc.tile_pool(name="psO", bufs=2, space="PSUM"))
    ypool = ctx.enter_context(tc.tile_pool(name="yp", bufs=3))
    y = aps["y"]
    ngp = GRP // 2

    def emit_outproj_tile(st):
        tsl = slice(st * P, (st + 1) * P)
        y_sb = ypool.tile([P, ndb, NB], bf16, tag="y")
        for db in range(ndb):
            dsl = slice(db * NB, (db + 1) * NB)
            ps_y = psO.tile([P, NB], f32, tag="y")
            for gp in range(ngp):
                jsl = slice(2 * gp, 2 * gp + 2)
                res_mm(ps_y, ot8hi[:, jsl, tsl], ot8lo[:, jsl, tsl],
                       wo_sb[:, jsl, 0, dsl], wo_sb[:, jsl, 1, dsl],
                       gp, gp == 0, gp == ngp - 1)
            nc.vector.tensor_scalar_mul(y_sb[:, db, :], ps_y,
                                        1.0 / (OSC * WSC))
            # split the final tile's store per-db so the tail is one small
            # transfer instead of a full 512KB store after the last matmul
            if st == nt - 1:
                nc.sync.dma_start(out=y[tsl, dsl], in_=y_sb[:, db, :])
        if st < nt - 1:
            nc.sync.dma_start(out=y[tsl, :], in_=y_sb)

    prev = None
    nst = 0
    for i, (g, qb) in enumerate(blocks):
        prev = emit_block(g, qb, prev)
        if i >= 5:
            emit_outproj_tile(i - 5)
            nst = i - 4

    # epilogue: drain last block (AV + rowsum + normalize); its tree was
    # already folded inside emit_block
    (pg, pqb, ppt8, ppt, pps_o, ptree, ptall, pacc) = prev
    for p in range(npair):
        if p < 2:
            psl = slice(2 * p, 2 * p + 2)
            nc.tensor.matmul(pps_o, lhsT=v8hi[:, psl, :], rhs=ppt8[:, psl, :],
                             start=(p == 0), stop=False, perf_mode=DR)
            nc.tensor.matmul(pps_o, lhsT=v8lo[:, psl, :], rhs=ppt8[:, psl, :],
                             start=False, stop=False, perf_mode=DR)
        else:
            nc.tensor.matmul(pps_o, lhsT=v_sb[:, 2 * p, :],
                             rhs=ppt[:, 2 * p - 4, :], start=False, stop=False)
            nc.tensor.matmul(pps_o, lhsT=v_sb[:, 2 * p + 1, :],
                             rhs=ppt[:, 2 * p - 3, :],
                             start=False, stop=(p == npair - 1))
    finish_norm(pg, pqb, pacc, pps_o)
    for st in range(nst, nt):
        emit_outproj_tile(st)


def build_program(s=S, d=D):
    import concourse.tile as tile
    from concourse import bacc, mybir

    nc = bacc.Bacc("TRN2", target_bir_lowering=False, debug=False)
    bf16 = mybir.dt.bfloat16
    f32 = mybir.dt.float32
    e4 = mybir.dt.float8e4
    names = {
        "xp": ([d, s // NB, 2, NB], e4),
        "wq": ([GRP, d, 2, DH], e4),
        "wkv": ([d, 4, DH], e4),
        "wo": ([EH, 2, d], e4),
        "bq": ([EH], f32), "bk": ([DH], f32), "bv": ([DH], f32),
    }
    aps = {k: nc.dram_tensor(k, sh, dt, kind="ExternalInput").ap()
           for k, (sh, dt) in names.items()}
    aps["y"] = nc.dram_tensor("y", [s, d], bf16, kind="ExternalOutput").ap()
    with tile.TileContext(nc) as tc:
        with ExitStack() as ctx:
            _emit(ctx, tc, aps, s=s, d=d)
    nc.compile()
    return nc


def _res_split(v32, scale):
    """fp8 residual split of v32*scale: hi = e4(v*scale), lo = e4(v*scale-hi)."""
    e4 = ml_dtypes.float8_e4m3
    vs = v32 * np.float32(scale)
    hi = vs.astype(e4)
    lo = (vs - hi.astype(np.float32)).astype(e4)
    return hi, lo


def make_in_maps(x, Wq, bq, Wk, bk, Wv, bv, Wo, bo):
    e4 = ml_dtypes.float8_e4m3
    in_maps = []
    xsplit = []
    for b in range(B):
        xT = np.ascontiguousarray(x[b].T)  # [D, S]
        hi, lo = _res_split(xT, 1.0)
        nsb = S // NB
        xp = np.empty((D, nsb, 2, NB), dtype=e4)
        xp[:, :, 0, :] = hi.reshape(D, nsb, NB)
        xp[:, :, 1, :] = lo.reshape(D, nsb, NB)
        xsplit.append(xp)
    for b in range(B):
        xp = xsplit[b]
        for h in range(KV):
            wqh, wql = _res_split(np.ascontiguousarray(
                Wq[:, h * EH:(h + 1) * EH]), WSC)
            wq = np.empty((GRP, D, 2, DH), dtype=e4)
            for g in range(GRP):
                wq[g, :, 0, :] = wqh[:, g * DH:(g + 1) * DH]
                wq[g, :, 1, :] = wql[:, g * DH:(g + 1) * DH]
            wkh, wkl = _res_split(np.ascontiguousarray(
                Wk[:, h * DH:(h + 1) * DH]), WSC)
            wvh, wvl = _res_split(np.ascontiguousarray(
                Wv[:, h * DH:(h + 1) * DH]), WSC)
            wkv = np.empty((D, 4, DH), dtype=e4)
            wkv[:, 0, :] = wkh
            wkv[:, 1, :] = wkl
            wkv[:, 2, :] = wvh
            wkv[:, 3, :] = wvl
            woh, wol = _res_split(np.ascontiguousarray(
                Wo[h * EH:(h + 1) * EH, :]), WSC)
            wo = np.empty((EH, 2, D), dtype=e4)
            wo[:, 0, :] = woh
            wo[:, 1, :] = wol
            in_maps.append({
                "xp": xp, "wq": wq, "wkv": wkv, "wo": wo,
                "bq": np.ascontiguousarray(bq[h * EH:(h + 1) * EH]).astype(np.float32),
                "bk": np.ascontiguousarray(bk[h * DH:(h + 1) * DH]).astype(np.float32),
                "bv": np.ascontiguousarray(bv[h * DH:(h + 1) * DH]).astype(np.float32),
            })
    return in_maps


_PROG = None


def _get_program():
    global _PROG
    if _PROG is None:
        _PROG = build_program()
    return _PROG


def run_cores(in_maps, trace=False, **kw):
    from concourse.bass_utils import run_bass_kernel_spmd
    nc = _get_program()
    return run_bass_kernel_spmd(nc, in_maps, list(range(8)), trace=trace, **kw)


def kernel(**inputs):
    x = np.asarray(inputs["x"], dtype=np.float32)
    Wq = np.asarray(inputs["Wq"], dtype=np.float32)
    bq = np.asarray(inputs["bq"], dtype=np.float32)
    Wk = np.asarray(inputs["Wk"], dtype=np.float32)
    bk = np.asarray(inputs["bk"], dtype=np.float32)
    Wv = np.asarray(inputs["Wv"], dtype=np.float32)
    bv = np.asarray(inputs["bv"], dtype=np.float32)
    Wo = np.asarray(inputs["Wo"], dtype=np.float32)
    bo = np.asarray(inputs["bo"], dtype=np.float32)

    in_maps = make_in_maps(x, Wq, bq, Wk, bk, Wv, bv, Wo, bo)
    res = run_cores(in_maps)
    out = np.empty((B, S, D), dtype=np.float32)
    for b in range(B):
        acc = res.results[b * KV]["y"].astype(np.float32)
        for h in range(1, KV):
            acc = acc + res.results[b * KV + h]["y"].astype(np.float32)
        out[b] = acc + bo[None, :]
    return out
